# revision 6
# baseline (speedup 1.0000x reference)
"""GPT2 causal attention (B=2, T=2048, C=1024, H=16) on 8 TRN2 NeuronCores.

Sharding: core g = (batch b = g//4, head-group hg = g%4 of 4 heads).
Tensor-parallel over heads (column-split W_attn, row-split W_proj) x
data-parallel over batch. Each core computes a full [T, C] partial of the
output projection for its 4 heads; host sums the 4 partials per batch and
adds b_proj. No collectives.

v2: t-banded QKV pipeline. x is packed band-major (4 bands of 512 t
positions, 8 c-chunks contiguous per band) so Q/K/V for band 0 complete
~6us in and the attention exp stream (the Act-engine bottleneck) starts
immediately, instead of waiting for a full-T QKV phase. Remaining QKV
bands and the dc1 (second head pair) projections are emitted as filler
groups inside the attention blocks to keep the PE busy while Act chews
exp. Diagonal tri-masks and output staging copies run on the otherwise
idle Pool engine (Act keeps only exp; DVE keeps evictions/normalize).
"""

import numpy as np
import ml_dtypes

BF16 = ml_dtypes.bfloat16

B, T, C, H, D = 2, 2048, 1024, 16, 64
HL = 4          # heads per core
DL = HL * D     # 256 local head dims
N_CORES = 8
NT = T // 128   # 16 tk tiles
NJ = T // 512   # 4 tq groups per head pair
NBAND = 4       # t bands of 512 for QKV
SCALE = 1.0 / np.sqrt(D)

_CACHE = {}


def _build_program():
    import concourse.tile as tile
    from concourse import bacc
    import concourse.mybir as mybir

    f32 = mybir.dt.float32
    f16 = mybir.dt.float16
    bf16 = mybir.dt.bfloat16
    Exp = mybir.ActivationFunctionType.Exp

    nc = bacc.Bacc("TRN2", target_bir_lowering=False, debug=False)

    # ---- DRAM I/O (host pre-sharded and pre-packed to SBUF layout) ----
    xT_d = nc.dram_tensor("xTp", [128, 16384], bf16, kind="ExternalInput").ap()
    wq_d = nc.dram_tensor("wqp", [128, 2048], bf16, kind="ExternalInput").ap()
    wk_d = nc.dram_tensor("wkp", [128, 2048], bf16, kind="ExternalInput").ap()
    wv_d = nc.dram_tensor("wvp", [128, 2048], bf16, kind="ExternalInput").ap()
    wp_d = nc.dram_tensor("wpp", [128, 2048], bf16, kind="ExternalInput").ap()
    m32_d = nc.dram_tensor("m32", [128, 260], f32, kind="ExternalInput").ap()
    mbf_d = nc.dram_tensor("mbf", [128, 256], bf16, kind="ExternalInput").ap()
    out_d = nc.dram_tensor("out", [T, C], f16, kind="ExternalOutput").ap()
    DBG = bool(__import__("os").environ.get("KDBG"))
    if DBG:
        qT_dump = nc.dram_tensor("qTd", [128, 2 * T], bf16, kind="ExternalOutput").ap()
        kT_dump = nc.dram_tensor("kTd", [128, 2 * T], bf16, kind="ExternalOutput").ap()
        yT_dump = nc.dram_tensor("yTd", [128, 2 * T], bf16, kind="ExternalOutput").ap()
        V_dump = nc.dram_tensor("Vd", [128, NT * HL * 65], bf16, kind="ExternalOutput").ap()

    with tile.TileContext(nc) as tc:
        with (
            tc.tile_pool(name="const", bufs=1) as cpool,
            tc.tile_pool(name="exp", bufs=8) as epool,
            tc.tile_pool(name="small", bufs=8) as spool,
            tc.tile_pool(name="ostage", bufs=6) as opool,
            tc.tile_pool(name="pssc", bufs=2, space="PSUM") as pool_sc,
            tc.tile_pool(name="psy", bufs=2, space="PSUM") as pool_yps,
            tc.tile_pool(name="pspt", bufs=2, space="PSUM") as pool_pt,
        ):
            # ---- persistent SBUF ----
            # xT band-major: band ts (512 t), chunk c at [:, ts*4096 + c*512]
            xT = cpool.tile([128, 16384], bf16, tag="xT")
            wq = cpool.tile([128, 2048], bf16, tag="wq")    # dc*1024 + c*128
            wk = cpool.tile([128, 2048], bf16, tag="wk")
            wv = cpool.tile([128, 2048], bf16, tag="wv")    # c*256
            wp = cpool.tile([128, 2048], bf16, tag="wp")    # dc*1024 + cols
            m32 = cpool.tile([128, 260], f32, tag="m32")    # bq|bk|bvr
            mbf = cpool.tile([128, 256], bf16, tag="mbf")   # tri|eye
            bq = m32[:, 0:2]
            bk = m32[:, 2:4]
            bvr = m32[:, 4:260]
            tri = mbf[:, 0:128]
            eye = mbf[:, 128:256]
            qT = cpool.tile([128, 2 * T], bf16, tag="qT")   # head h: [64*(h%2):, (h//2)*T + t]
            kT = cpool.tile([128, 2 * T], bf16, tag="kT")
            yT = cpool.tile([128, 2 * T], bf16, tag="yT")   # pair hp at [:, hp*T + t]
            # V_aug bf16, head-major: slot (h,tt) at [:, h*NT*65 + tt*65 : +65],
            # col 64 = ones (so A*V also yields the softmax row-sums).
            V = cpool.tile([128, HL * NT * 65], bf16, tag="V")

            Vv = V[:, :].rearrange("p (h t e) -> p h t e", h=HL, t=NT)
            nc.vector.memset(Vv[:, :, :, 64:65], 1.0)

            # ---- load inputs; transfers serialize on the DMA engine so
            # order = arrival order. Band 0 + dc0 weights first.
            nc.sync.dma_start(out=wq[:, 0:1024], in_=wq_d[:, 0:1024])
            nc.sync.dma_start(out=wk[:, 0:1024], in_=wk_d[:, 0:1024])
            nc.sync.dma_start(out=m32[:, :], in_=m32_d[:, :])
            nc.sync.dma_start(out=mbf[:, :], in_=mbf_d[:, :])

            def dma_band(ts):
                for p in range(2):
                    lo = ts * 4096 + p * 2048
                    nc.sync.dma_start(out=xT[:, lo:lo + 2048],
                                      in_=xT_d[:, lo:lo + 2048])

            dma_band(0)
            nc.sync.dma_start(out=wv[:, :], in_=wv_d[:, :])
            dma_band(1)
            nc.sync.dma_start(out=wq[:, 1024:2048], in_=wq_d[:, 1024:2048])
            nc.sync.dma_start(out=wk[:, 1024:2048], in_=wk_d[:, 1024:2048])
            dma_band(2)
            nc.sync.dma_start(out=wp[:, :], in_=wp_d[:, :])
            dma_band(3)

            # ---- QKV band groups ----
            def emit_qk_band(w_sb, b_sb, dst, dc, ts):
                ps = pool_pt.tile([128, 512], f32, tag="pt",
                                  name=f"qk{dc}_{ts}")
                for c in range(8):
                    nc.tensor.matmul(
                        ps[:, :],
                        w_sb[:, dc * 1024 + c * 128: dc * 1024 + (c + 1) * 128],
                        xT[:, ts * 4096 + c * 512: ts * 4096 + (c + 1) * 512],
                        start=(c == 0), stop=(c == 7),
                    )
                nc.vector.tensor_scalar_add(
                    dst[:, dc * T + ts * 512: dc * T + (ts + 1) * 512],
                    ps[:, :], b_sb[:, dc:dc + 1],
                )

            def emit_v_tile(tt):
                ts, k = divmod(tt, 4)
                ps = pool_pt.tile([128, DL], f32, tag="pt", name=f"vps{tt}")
                for c in range(8):
                    nc.tensor.matmul(
                        ps[:, :],
                        xT[:, ts * 4096 + c * 512 + k * 128:
                           ts * 4096 + c * 512 + (k + 1) * 128],
                        wv[:, c * DL:(c + 1) * DL],
                        start=(c == 0), stop=(c == 7),
                    )
                nc.vector.tensor_add(
                    Vv[:, :, tt, 0:64],
                    ps[:, :].rearrange("p (h e) -> p h e", h=HL),
                    bvr[:, :].rearrange("p (h e) -> p h e", h=HL),
                )

            # band 0 inline: K first (scores need kT tile0 + qT block0)
            emit_qk_band(wk, bk, kT, 0, 0)
            emit_qk_band(wq, bq, qT, 0, 0)
            for tt in range(4):
                emit_v_tile(tt)

            # filler groups: bands 1-3 dc0 QKV, then dc1 Q/K bands.
            fillers = []
            for ts in (1, 2, 3):
                fillers.append(lambda ts=ts: emit_qk_band(wk, bk, kT, 0, ts))
                fillers.append(lambda ts=ts: emit_qk_band(wq, bq, qT, 0, ts))
                for k in range(4):
                    fillers.append(lambda tt=4 * ts + k: emit_v_tile(tt))
            for ts in range(4):
                fillers.append(lambda ts=ts: emit_qk_band(wk, bk, kT, 1, ts))
                fillers.append(lambda ts=ts: emit_qk_band(wq, bq, qT, 1, ts))
            fillers.reverse()  # pop() from the front
            # groups that must be emitted before attn block (hp, j) starts
            need_before = {(0, 1): 6, (0, 2): 12, (0, 3): 18, (1, 0): 26}
            n_popped = [0]

            def pop_filler():
                if fillers:
                    fillers.pop()()
                    n_popped[0] += 1

            # ---- attention ----
            def emit_proj(tt):
                # out[tt band, :] = sum_dc yT[dc, tt]^T @ wp[dc]; fp16 partial out
                for cc in range(2):
                    pp = pool_pt.tile([128, 512], f32, tag="pt", name=f"pp{tt}_{cc}")
                    for dc in range(2):
                        nc.tensor.matmul(
                            pp[:, :],
                            yT[:, dc * T + tt * 128: dc * T + (tt + 1) * 128],
                            wp[:, dc * C + cc * 512: dc * C + (cc + 1) * 512],
                            start=(dc == 0), stop=(dc == 1),
                        )
                    ot = opool.tile([128, 512], f16, tag="ot", name=f"ot{tt}_{cc}")
                    if (tt + cc) % 2 == 0:
                        nc.scalar.copy(ot[:, :], pp[:, :])
                    else:
                        nc.vector.tensor_copy(ot[:, :], pp[:, :])
                    nc.sync.dma_start(
                        out=out_d[tt * 128:(tt + 1) * 128, cc * 512:(cc + 1) * 512],
                        in_=ot[:, :],
                    )

            def emit_completion(hp, j, q4, ytiles):
                # q-chunk jj = 4j+q4 finished accumulating: normalize both
                # heads' [128 q, 64] + denominators (col 64 of each slot),
                # transpose to yT layout, then (hp==1) project that band.
                jj = 4 * j + q4
                yt = ytiles[q4 // 2]
                base = 132 * (q4 % 2)
                dn = yt[:, :].rearrange("p (s e) -> p s e", s=4)[
                    :, 2 * (q4 % 2):2 * (q4 % 2) + 2, 64]
                rc = spool.tile([128, 2], f32, tag="rc", name=f"rc{hp}_{jj}")
                nc.vector.reciprocal(rc[:, :], dn)
                yp = spool.tile([128, 128], bf16, tag="yp", name=f"yp{hp}_{jj}")
                for half in range(2):
                    nc.vector.tensor_scalar_mul(
                        yp[:, half * 64:(half + 1) * 64],
                        yt[:, base + half * 66: base + half * 66 + 64],
                        rc[:, half:half + 1],
                    )
                tp = pool_pt.tile([128, 128], bf16, tag="pt", name=f"tp{hp}_{jj}")
                nc.tensor.transpose(tp[:, :], yp[:, :], eye[:, :])
                nc.vector.tensor_copy(yT[:, hp * T + jj * 128: hp * T + (jj + 1) * 128],
                                      tp[:, :])
                if hp == 1:
                    emit_proj(jj)

            def attn_block(hp, j, do_fill):
                fb = hp * T
                ni = 4 * j + 4
                # yps slot (q4, half) = 2*q4+half: slots 0-3 in ya, 4-7 in yb;
                # 66 cols each (65 used: col 64 = softmax denominator).
                ytiles = [
                    pool_yps.tile([128, 264], f32, tag="yps", name=f"y{hp}_{j}_{m}")
                    for m in range(2)
                ]
                ets = [None] * ni

                def emit_score(i):
                    d0 = max(128 * (i - 4 * j), 0)
                    sc = pool_sc.tile([128, 1024], f32, tag="sc",
                                      name=f"sc{hp}_{j}_{i}")
                    for half in range(2):
                        po = 64 * half
                        nc.tensor.matmul(
                            sc[:, half * 512 + d0:(half + 1) * 512],
                            kT[po:po + 64, fb + i * 128: fb + (i + 1) * 128],
                            qT[po:po + 64, fb + j * 512 + d0: fb + (j + 1) * 512],
                            start=True, stop=True,
                        )
                    et = epool.tile([128, 1024], bf16, tag="exp",
                                    name=f"et{hp}_{j}_{i}")
                    et2 = et[:, :].rearrange("p (g q) -> p g q", g=2)
                    sc2 = sc[:, :].rearrange("p (g q) -> p g q", g=2)
                    nc.scalar.activation(
                        et2[:, :, d0:512], sc2[:, :, d0:512], Exp,
                        scale=float(SCALE),
                    )
                    if i >= 4 * j:  # diagonal chunk: causal mask (post-exp)
                        for half in range(2):
                            sl = slice(half * 512 + d0, half * 512 + d0 + 128)
                            nc.gpsimd.tensor_mul(et[:, sl], et[:, sl], tri[:, :])
                    ets[i] = et

                def emit_av(i):
                    # PSUM start_tensor_calc marks the whole 2KB bank pending-
                    # zero, so: ONE start per yps bank (its first matmul); the
                    # other slots' first writes land on pending-zero bytes and
                    # overwrite; ONE stop on the bank's last matmul.
                    et = ets[i]
                    for half in range(2):
                        h = 2 * hp + half
                        for q4 in range(4):
                            if 4 * j + q4 < i:
                                continue
                            s = 2 * q4 + half
                            yt = ytiles[s // 4]
                            off = (s % 4) * 66
                            bank_start = (i == 0 and half == 0 and q4 % 2 == 0)
                            bank_stop = (half == 1 and q4 % 2 == 1
                                         and i == 4 * j + q4)
                            nc.tensor.matmul(
                                yt[:, off:off + 65],
                                et[:, half * 512 + q4 * 128: half * 512 + (q4 + 1) * 128],
                                Vv[:, h, i, :],
                                start=bank_start, stop=bank_stop,
                                skip_group_check=True,
                            )

                # 1-deep software pipeline: score(i+1) issues before av(i) so
                # the PE never waits on Act's exp(i).
                for i in range(ni + 1):
                    if i < ni:
                        emit_score(i)
                    if i >= 1:
                        emit_av(i - 1)
                        if (i - 1) >= 4 * j:
                            emit_completion(hp, j, (i - 1) - 4 * j, ytiles)
                        if do_fill and (i % 2 == 0):
                            pop_filler()

            for hp in range(2):
                for j in range(NJ):
                    req = need_before.get((hp, j), 0)
                    while n_popped[0] < req:
                        pop_filler()
                    attn_block(hp, j, do_fill=True)
            while fillers:
                pop_filler()

            if DBG:
                nc.sync.dma_start(out=qT_dump[:, :], in_=qT[:, :])
                nc.sync.dma_start(out=kT_dump[:, :], in_=kT[:, :])
                nc.sync.dma_start(out=yT_dump[:, :], in_=yT[:, :])
                nc.sync.dma_start(out=V_dump[:, :], in_=V[:, :])

    nc.compile()
    return nc


def get_program():
    if "nc" not in _CACHE:
        _CACHE["nc"] = _build_program()
    return _CACHE["nc"]


def _pack_cmajor(a):
    """[C_rows, N] -> [128, (C_rows/128)*N] with chunk c at [:, c*N:(c+1)*N]."""
    rows, n = a.shape
    return np.ascontiguousarray(
        a.reshape(rows // 128, 128, n).transpose(1, 0, 2).reshape(128, -1))


def _pack_banded(a):
    """x[b].T [1024, 2048] -> [128, 16384], col = ts*4096 + c*512 + t'."""
    return np.ascontiguousarray(
        a.reshape(8, 128, 4, 512).transpose(1, 2, 0, 3).reshape(128, 16384))


def make_in_maps(x, W_attn, b_attn, W_proj):
    """Host-side sharding: per-core input dict."""
    x = np.asarray(x, np.float32)
    W_attn = np.asarray(W_attn, np.float32)
    b_attn = np.asarray(b_attn, np.float32)
    W_proj = np.asarray(W_proj, np.float32)

    tk = np.arange(128)[:, None]
    tq = np.arange(128)[None, :]
    tri = (tq >= tk).astype(BF16)
    eye = np.eye(128, dtype=BF16)
    mbf = np.ascontiguousarray(np.concatenate([tri, eye], axis=1))

    xT_b = [_pack_banded(x[b].T.astype(BF16)) for b in range(B)]

    def _pack_dcmajor(a):
        # [1024, 256] -> [128, dc*1024 + c*128]
        return np.concatenate(
            [_pack_cmajor(a[:, 0:128]), _pack_cmajor(a[:, 128:256])], axis=1)

    in_maps = []
    for g in range(N_CORES):
        b, hg = divmod(g, 4)
        cs = slice(hg * DL, (hg + 1) * DL)
        wq = _pack_dcmajor(W_attn[:, 0 * C:1 * C][:, cs].astype(BF16))
        wk = _pack_dcmajor(W_attn[:, 1 * C:2 * C][:, cs].astype(BF16))
        wv = _pack_cmajor(W_attn[:, 2 * C:3 * C][:, cs].astype(BF16))
        wp = _pack_cmajor(W_proj[cs, :].astype(BF16))
        bq = np.ascontiguousarray(b_attn[0 * C:1 * C][cs].reshape(2, 128).T)
        bk = np.ascontiguousarray(b_attn[1 * C:2 * C][cs].reshape(2, 128).T)
        bvr = np.tile(b_attn[2 * C:3 * C][cs][None, :], (128, 1))
        m32 = np.ascontiguousarray(
            np.concatenate([bq, bk, bvr], axis=1).astype(np.float32))
        in_maps.append({
            "xTp": xT_b[b],
            "wqp": wq, "wkp": wk, "wvp": wv, "wpp": wp,
            "m32": m32, "mbf": mbf,
        })
    return in_maps


def assemble_output(results, b_proj):
    """results: per-core dicts with 'out' [T, C] fp16 partials."""
    b_proj = np.asarray(b_proj, np.float32)
    out = np.zeros((B, T, C), np.float32)
    for g in range(N_CORES):
        out[g // 4] += np.asarray(results[g]["out"], np.float32)
    out += b_proj[None, None, :]
    return out


def kernel(x, W_attn, b_attn, W_proj, b_proj):
    from concourse.bass_utils import run_bass_kernel_spmd

    nc = get_program()
    in_maps = make_in_maps(x, W_attn, b_attn, W_proj)
    res = run_bass_kernel_spmd(nc, in_maps, list(range(N_CORES)))
    return assemble_output(res.results, b_proj)


# revision 7
# speedup vs baseline: 1.0047x; 1.0047x over previous
"""GPT2 causal attention (B=2, T=2048, C=1024, H=16) on 8 TRN2 NeuronCores.

Sharding: core g = (batch b = g//4, head-group hg = g%4 of 4 heads).
Tensor-parallel over heads (column-split W_attn, row-split W_proj) x
data-parallel over batch. Each core computes a full [T, C] partial of the
output projection for its 4 heads; host sums the 4 partials per batch and
adds b_proj. No collectives.

v3: fp8 DoubleRow QKV + t-banded pipeline.
  - QKV projections run as fp8e4m3 DoubleRow matmuls (K=256 per pass, 0.5
    PE cycles/row = 4x bf16 throughput). Accuracy is preserved by hi+lo
    error compensation: host splits x and 32*W into fp8 hi + fp8 residual;
    x@W = xh@wh + xh@wl + xl@wh (+O(2^-8) dropped xl@wl), i.e. ~bf16-level
    error at 6/8 the bf16 cycle cost. The 32x weight scale (fp8 subnormal
    avoidance) is folded into the exp scale (1/(8*1024), scores carry
    32*32) and the V ones-column (32.0, so the softmax denominator scales
    with the 32x-scaled y numerator and the reciprocal-normalize cancels
    everything exactly).
  - x is packed band-major (4 bands of 512 t, 8 c-chunks contiguous per
    band) so band-0 QKV completes ~6.5us in and the attention exp stream
    (Act is the #2 engine) starts immediately; remaining bands + dc1
    weights are filler groups inside the attention blocks.
  - Attention blocks interleave head pairs (0,j),(1,j) so the output
    projection and its DMA flow evenly instead of bunching in the tail.
  - Diagonal tri-masks run on the otherwise idle Pool engine.
"""

import numpy as np
import ml_dtypes

BF16 = ml_dtypes.bfloat16
F8 = ml_dtypes.float8_e4m3fn

B, T, C, H, D = 2, 2048, 1024, 16, 64
HL = 4          # heads per core
DL = HL * D     # 256 local head dims
N_CORES = 8
NT = T // 128   # 16 tk tiles
NJ = T // 512   # 4 tq groups per head pair
SCALE = 1.0 / np.sqrt(D)
WS = 32.0       # host-side weight pre-scale (fp8 subnormal avoidance)

_CACHE = {}


def _build_program():
    import concourse.tile as tile
    from concourse import bacc
    import concourse.mybir as mybir

    f32 = mybir.dt.float32
    f16 = mybir.dt.float16
    bf16 = mybir.dt.bfloat16
    fp8 = mybir.dt.float8e4
    DR = mybir.MatmulPerfMode.DoubleRow
    Exp = mybir.ActivationFunctionType.Exp

    nc = bacc.Bacc("TRN2", target_bir_lowering=False, debug=False)

    # ---- DRAM I/O (host pre-sharded and pre-packed to SBUF layout) ----
    xh_d = nc.dram_tensor("xhp", [128, 16384], fp8, kind="ExternalInput").ap()
    xl_d = nc.dram_tensor("xlp", [128, 16384], fp8, kind="ExternalInput").ap()
    wqh_d = nc.dram_tensor("wqh", [128, 2048], fp8, kind="ExternalInput").ap()
    wql_d = nc.dram_tensor("wql", [128, 2048], fp8, kind="ExternalInput").ap()
    wkh_d = nc.dram_tensor("wkh", [128, 2048], fp8, kind="ExternalInput").ap()
    wkl_d = nc.dram_tensor("wkl", [128, 2048], fp8, kind="ExternalInput").ap()
    wvh_d = nc.dram_tensor("wvh", [128, 2048], fp8, kind="ExternalInput").ap()
    wvl_d = nc.dram_tensor("wvl", [128, 2048], fp8, kind="ExternalInput").ap()
    wp_d = nc.dram_tensor("wpp", [128, 2048], bf16, kind="ExternalInput").ap()
    m32_d = nc.dram_tensor("m32", [128, 260], f32, kind="ExternalInput").ap()
    mbf_d = nc.dram_tensor("mbf", [128, 256], bf16, kind="ExternalInput").ap()
    out_d = nc.dram_tensor("out", [T, C], f16, kind="ExternalOutput").ap()
    DBG = bool(__import__("os").environ.get("KDBG"))
    if DBG:
        qT_dump = nc.dram_tensor("qTd", [128, 2 * T], bf16, kind="ExternalOutput").ap()
        kT_dump = nc.dram_tensor("kTd", [128, 2 * T], bf16, kind="ExternalOutput").ap()
        yT_dump = nc.dram_tensor("yTd", [128, 2 * T], bf16, kind="ExternalOutput").ap()
        V_dump = nc.dram_tensor("Vd", [128, NT * HL * 65], bf16, kind="ExternalOutput").ap()

    with tile.TileContext(nc) as tc:
        with (
            tc.tile_pool(name="const", bufs=1) as cpool,
            tc.tile_pool(name="exp", bufs=8) as epool,
            tc.tile_pool(name="small", bufs=8) as spool,
            tc.tile_pool(name="ostage", bufs=6) as opool,
            tc.tile_pool(name="pssc", bufs=2, space="PSUM") as pool_sc,
            tc.tile_pool(name="psy", bufs=2, space="PSUM") as pool_yps,
            tc.tile_pool(name="pspt", bufs=2, space="PSUM") as pool_pt,
        ):
            # ---- persistent SBUF ----
            # x band-major: band ts (512 t), chunk c at [:, ts*4096 + c*512]
            xh = cpool.tile([128, 16384], fp8, tag="xh")
            xl = cpool.tile([128, 16384], fp8, tag="xl")
            wqh = cpool.tile([128, 2048], fp8, tag="wqh")   # dc*1024 + c*128
            wql = cpool.tile([128, 2048], fp8, tag="wql")
            wkh = cpool.tile([128, 2048], fp8, tag="wkh")
            wkl = cpool.tile([128, 2048], fp8, tag="wkl")
            wvh = cpool.tile([128, 2048], fp8, tag="wvh")   # c*256
            wvl = cpool.tile([128, 2048], fp8, tag="wvl")
            wp = cpool.tile([128, 2048], bf16, tag="wp")    # dc*1024 + cols
            m32 = cpool.tile([128, 260], f32, tag="m32")    # bq|bk|bvr (x32)
            mbf = cpool.tile([128, 256], bf16, tag="mbf")   # tri|eye
            bq = m32[:, 0:2]
            bk = m32[:, 2:4]
            bvr = m32[:, 4:260]
            tri = mbf[:, 0:128]
            eye = mbf[:, 128:256]
            qT = cpool.tile([128, 2 * T], bf16, tag="qT")   # head h: [64*(h%2):, (h//2)*T + t]
            kT = cpool.tile([128, 2 * T], bf16, tag="kT")
            yT = cpool.tile([128, 2 * T], bf16, tag="yT")   # pair hp at [:, hp*T + t]
            # V_aug bf16, head-major: slot (h,tt) at [:, h*NT*65 + tt*65 : +65],
            # col 64 = 32.0 (so A*V yields 32x softmax row-sums, matching the
            # 32x-scaled y numerator; the reciprocal-normalize cancels both).
            V = cpool.tile([128, HL * NT * 65], bf16, tag="V")

            Vv = V[:, :].rearrange("p (h t e) -> p h t e", h=HL, t=NT)
            nc.vector.memset(Vv[:, :, :, 64:65], float(WS))

            # ---- load inputs; transfers serialize on the DMA engine so
            # order = arrival order. Band 0 + dc0 weights first.
            nc.sync.dma_start(out=wqh[:, 0:1024], in_=wqh_d[:, 0:1024])
            nc.sync.dma_start(out=wql[:, 0:1024], in_=wql_d[:, 0:1024])
            nc.sync.dma_start(out=wkh[:, 0:1024], in_=wkh_d[:, 0:1024])
            nc.sync.dma_start(out=wkl[:, 0:1024], in_=wkl_d[:, 0:1024])
            nc.sync.dma_start(out=m32[:, :], in_=m32_d[:, :])

            def dma_band(ts):
                for p in range(2):
                    lo = ts * 4096 + p * 2048
                    nc.sync.dma_start(out=xh[:, lo:lo + 2048],
                                      in_=xh_d[:, lo:lo + 2048])
                    nc.sync.dma_start(out=xl[:, lo:lo + 2048],
                                      in_=xl_d[:, lo:lo + 2048])

            dma_band(0)
            nc.sync.dma_start(out=mbf[:, :], in_=mbf_d[:, :])
            nc.sync.dma_start(out=wvh[:, :], in_=wvh_d[:, :])
            nc.sync.dma_start(out=wvl[:, :], in_=wvl_d[:, :])
            dma_band(1)
            nc.sync.dma_start(out=wqh[:, 1024:2048], in_=wqh_d[:, 1024:2048])
            nc.sync.dma_start(out=wql[:, 1024:2048], in_=wql_d[:, 1024:2048])
            nc.sync.dma_start(out=wkh[:, 1024:2048], in_=wkh_d[:, 1024:2048])
            nc.sync.dma_start(out=wkl[:, 1024:2048], in_=wkl_d[:, 1024:2048])
            dma_band(2)
            nc.sync.dma_start(out=wp[:, :], in_=wp_d[:, :])
            dma_band(3)

            # ---- QKV band groups: fp8 DoubleRow, hi/lo compensated ----
            def qk_pair_mms(ps, wh, wl, dc, ts, a, start, stop):
                # one K=256 pair (c = 2a, 2a+1): xh@wh + xh@wl + xl@wh
                wb = dc * 1024 + 256 * a
                xb = ts * 4096 + 1024 * a
                lh = wh[:, wb:wb + 256].rearrange("p (k m) -> p k m", k=2)
                ll = wl[:, wb:wb + 256].rearrange("p (k m) -> p k m", k=2)
                rh = xh[:, xb:xb + 1024].rearrange("p (k t) -> p k t", k=2)
                rl = xl[:, xb:xb + 1024].rearrange("p (k t) -> p k t", k=2)
                nc.tensor.matmul(ps[:, :], lh, rh, start=start, stop=False,
                                 perf_mode=DR)
                nc.tensor.matmul(ps[:, :], ll, rh, start=False, stop=False,
                                 perf_mode=DR)
                nc.tensor.matmul(ps[:, :], lh, rl, start=False, stop=stop,
                                 perf_mode=DR)

            def emit_qk_band(wh, wl, b_sb, dst, dc, ts):
                ps = pool_pt.tile([128, 512], f32, tag="pt",
                                  name=f"qk{dc}_{ts}")
                for a in range(4):
                    qk_pair_mms(ps, wh, wl, dc, ts, a, a == 0, a == 3)
                nc.vector.tensor_scalar_add(
                    dst[:, dc * T + ts * 512: dc * T + (ts + 1) * 512],
                    ps[:, :], b_sb[:, dc:dc + 1],
                )

            def emit_qk_band0():
                # band 0 Q/K interleaved per pair (chases the arriving DMA)
                psk = pool_pt.tile([128, 512], f32, tag="pt", name="qk0_k0")
                psq = pool_pt.tile([128, 512], f32, tag="pt", name="qk0_q0")
                for a in range(4):
                    qk_pair_mms(psk, wkh, wkl, 0, 0, a, a == 0, a == 3)
                    qk_pair_mms(psq, wqh, wql, 0, 0, a, a == 0, a == 3)
                nc.vector.tensor_scalar_add(kT[:, 0:512], psk[:, :], bk[:, 0:1])
                nc.vector.tensor_scalar_add(qT[:, 0:512], psq[:, :], bq[:, 0:1])

            def emit_v_tile(tt):
                ts, k = divmod(tt, 4)
                ps = pool_pt.tile([128, DL], f32, tag="pt", name=f"vps{tt}")
                xhb = xh[:, ts * 4096:(ts + 1) * 4096].rearrange(
                    "p (c t) -> p c t", c=8)
                xlb = xl[:, ts * 4096:(ts + 1) * 4096].rearrange(
                    "p (c t) -> p c t", c=8)
                for a in range(4):
                    lh = xhb[:, 2 * a:2 * a + 2, k * 128:(k + 1) * 128]
                    ll = xlb[:, 2 * a:2 * a + 2, k * 128:(k + 1) * 128]
                    rh = wvh[:, 512 * a:512 * a + 512].rearrange(
                        "p (k e) -> p k e", k=2)
                    rl = wvl[:, 512 * a:512 * a + 512].rearrange(
                        "p (k e) -> p k e", k=2)
                    nc.tensor.matmul(ps[:, :], lh, rh, start=(a == 0),
                                     stop=False, perf_mode=DR)
                    nc.tensor.matmul(ps[:, :], ll, rh, start=False,
                                     stop=False, perf_mode=DR)
                    nc.tensor.matmul(ps[:, :], lh, rl, start=False,
                                     stop=(a == 3), perf_mode=DR)
                nc.vector.tensor_add(
                    Vv[:, :, tt, 0:64],
                    ps[:, :].rearrange("p (h e) -> p h e", h=HL),
                    bvr[:, :].rearrange("p (h e) -> p h e", h=HL),
                )

            emit_qk_band0()
            for tt in range(4):
                emit_v_tile(tt)

            # filler groups, in dependency order for interleaved blocks:
            # dc0 band ts + V band ts must precede block (0,ts); dc1 band ts
            # must precede block (1,ts).
            fillers = []
            cum = {}
            for ts in range(4):
                if ts > 0:
                    fillers.append(
                        lambda ts=ts: emit_qk_band(wkh, wkl, bk, kT, 0, ts))
                    fillers.append(
                        lambda ts=ts: emit_qk_band(wqh, wql, bq, qT, 0, ts))
                    for k in range(4):
                        fillers.append(lambda tt=4 * ts + k: emit_v_tile(tt))
                    cum[(0, ts)] = len(fillers)
                fillers.append(
                    lambda ts=ts: emit_qk_band(wkh, wkl, bk, kT, 1, ts))
                fillers.append(
                    lambda ts=ts: emit_qk_band(wqh, wql, bq, qT, 1, ts))
                cum[(1, ts)] = len(fillers)
            fillers.reverse()  # pop() from the front
            n_popped = [0]

            def pop_filler():
                if fillers:
                    fillers.pop()()
                    n_popped[0] += 1

            # ---- attention ----
            def emit_proj(tt):
                # out[tt band, :] = sum_dc yT[dc, tt]^T @ wp[dc]; fp16 partial out
                for cc in range(2):
                    pp = pool_pt.tile([128, 512], f32, tag="pt", name=f"pp{tt}_{cc}")
                    for dc in range(2):
                        nc.tensor.matmul(
                            pp[:, :],
                            yT[:, dc * T + tt * 128: dc * T + (tt + 1) * 128],
                            wp[:, dc * C + cc * 512: dc * C + (cc + 1) * 512],
                            start=(dc == 0), stop=(dc == 1),
                        )
                    ot = opool.tile([128, 512], f16, tag="ot", name=f"ot{tt}_{cc}")
                    if (tt + cc) % 2 == 0:
                        nc.scalar.copy(ot[:, :], pp[:, :])
                    else:
                        nc.vector.tensor_copy(ot[:, :], pp[:, :])
                    nc.sync.dma_start(
                        out=out_d[tt * 128:(tt + 1) * 128, cc * 512:(cc + 1) * 512],
                        in_=ot[:, :],
                    )

            def emit_completion(hp, j, q4, ytiles):
                # q-chunk jj = 4j+q4 finished accumulating: normalize both
                # heads' [128 q, 64] + denominators (col 64 of each slot),
                # transpose to yT layout, then (hp==1) project that band.
                jj = 4 * j + q4
                yt = ytiles[q4 // 2]
                base = 132 * (q4 % 2)
                dn = yt[:, :].rearrange("p (s e) -> p s e", s=4)[
                    :, 2 * (q4 % 2):2 * (q4 % 2) + 2, 64]
                rc = spool.tile([128, 2], f32, tag="rc", name=f"rc{hp}_{jj}")
                nc.vector.reciprocal(rc[:, :], dn)
                yp = spool.tile([128, 128], bf16, tag="yp", name=f"yp{hp}_{jj}")
                for half in range(2):
                    nc.vector.tensor_scalar_mul(
                        yp[:, half * 64:(half + 1) * 64],
                        yt[:, base + half * 66: base + half * 66 + 64],
                        rc[:, half:half + 1],
                    )
                tp = pool_pt.tile([128, 128], bf16, tag="pt", name=f"tp{hp}_{jj}")
                nc.tensor.transpose(tp[:, :], yp[:, :], eye[:, :])
                nc.vector.tensor_copy(yT[:, hp * T + jj * 128: hp * T + (jj + 1) * 128],
                                      tp[:, :])
                if hp == 1:
                    emit_proj(jj)

            def attn_block(hp, j):
                fb = hp * T
                ni = 4 * j + 4
                # yps slot (q4, half) = 2*q4+half: slots 0-3 in ya, 4-7 in yb;
                # 66 cols each (65 used: col 64 = softmax denominator).
                ytiles = [
                    pool_yps.tile([128, 264], f32, tag="yps", name=f"y{hp}_{j}_{m}")
                    for m in range(2)
                ]
                ets = [None] * ni

                def emit_score(i):
                    d0 = max(128 * (i - 4 * j), 0)
                    sc = pool_sc.tile([128, 1024], f32, tag="sc",
                                      name=f"sc{hp}_{j}_{i}")
                    for half in range(2):
                        po = 64 * half
                        nc.tensor.matmul(
                            sc[:, half * 512 + d0:(half + 1) * 512],
                            kT[po:po + 64, fb + i * 128: fb + (i + 1) * 128],
                            qT[po:po + 64, fb + j * 512 + d0: fb + (j + 1) * 512],
                            start=True, stop=True,
                        )
                    et = epool.tile([128, 1024], bf16, tag="exp",
                                    name=f"et{hp}_{j}_{i}")
                    et2 = et[:, :].rearrange("p (g q) -> p g q", g=2)
                    sc2 = sc[:, :].rearrange("p (g q) -> p g q", g=2)
                    nc.scalar.activation(
                        et2[:, :, d0:512], sc2[:, :, d0:512], Exp,
                        scale=float(SCALE / (WS * WS)),
                    )
                    if i >= 4 * j:  # diagonal chunk: causal mask (post-exp)
                        for half in range(2):
                            sl = slice(half * 512 + d0, half * 512 + d0 + 128)
                            nc.gpsimd.tensor_mul(et[:, sl], et[:, sl], tri[:, :])
                    ets[i] = et

                def emit_av(i):
                    # PSUM start_tensor_calc marks the whole 2KB bank pending-
                    # zero, so: ONE start per yps bank (its first matmul); the
                    # other slots' first writes land on pending-zero bytes and
                    # overwrite; ONE stop on the bank's last matmul.
                    et = ets[i]
                    for half in range(2):
                        h = 2 * hp + half
                        for q4 in range(4):
                            if 4 * j + q4 < i:
                                continue
                            s = 2 * q4 + half
                            yt = ytiles[s // 4]
                            off = (s % 4) * 66
                            bank_start = (i == 0 and half == 0 and q4 % 2 == 0)
                            bank_stop = (half == 1 and q4 % 2 == 1
                                         and i == 4 * j + q4)
                            nc.tensor.matmul(
                                yt[:, off:off + 65],
                                et[:, half * 512 + q4 * 128: half * 512 + (q4 + 1) * 128],
                                Vv[:, h, i, :],
                                start=bank_start, stop=bank_stop,
                                skip_group_check=True,
                            )

                # 1-deep software pipeline: score(i+1) issues before av(i) so
                # the PE never waits on Act's exp(i).
                for i in range(ni + 1):
                    if i < ni:
                        emit_score(i)
                    if i >= 1:
                        emit_av(i - 1)
                        if (i - 1) >= 4 * j:
                            emit_completion(hp, j, (i - 1) - 4 * j, ytiles)
                        if i % 2 == 0:
                            pop_filler()

            for j in range(NJ):
                for hp in range(2):
                    req = cum.get((hp, j), 0)
                    while n_popped[0] < req:
                        pop_filler()
                    attn_block(hp, j)
            while fillers:
                pop_filler()

            if DBG:
                nc.sync.dma_start(out=qT_dump[:, :], in_=qT[:, :])
                nc.sync.dma_start(out=kT_dump[:, :], in_=kT[:, :])
                nc.sync.dma_start(out=yT_dump[:, :], in_=yT[:, :])
                nc.sync.dma_start(out=V_dump[:, :], in_=V[:, :])

    nc.compile()
    return nc


def get_program():
    if "nc" not in _CACHE:
        _CACHE["nc"] = _build_program()
    return _CACHE["nc"]


def _pack_cmajor(a):
    """[C_rows, N] -> [128, (C_rows/128)*N] with chunk c at [:, c*N:(c+1)*N]."""
    rows, n = a.shape
    return np.ascontiguousarray(
        a.reshape(rows // 128, 128, n).transpose(1, 0, 2).reshape(128, -1))


def _pack_banded(a):
    """x[b].T [1024, 2048] -> [128, 16384], col = ts*4096 + c*512 + t'."""
    return np.ascontiguousarray(
        a.reshape(8, 128, 4, 512).transpose(1, 2, 0, 3).reshape(128, 16384))


def _split_fp8(a):
    """a (f32) -> (hi, lo) fp8e4m3 with hi + lo ~= a (error ~2^-8 rel)."""
    hi = a.astype(F8)
    lo = (a - hi.astype(np.float32)).astype(F8)
    return hi, lo


def make_in_maps(x, W_attn, b_attn, W_proj):
    """Host-side sharding: per-core input dict."""
    x = np.asarray(x, np.float32)
    W_attn = np.asarray(W_attn, np.float32)
    b_attn = np.asarray(b_attn, np.float32)
    W_proj = np.asarray(W_proj, np.float32)

    tk = np.arange(128)[:, None]
    tq = np.arange(128)[None, :]
    tri = (tq >= tk).astype(BF16)
    eye = np.eye(128, dtype=BF16)
    mbf = np.ascontiguousarray(np.concatenate([tri, eye], axis=1))

    xhl_b = []
    for b in range(B):
        hi, lo = _split_fp8(x[b].T)
        xhl_b.append((_pack_banded(hi), _pack_banded(lo)))

    def _pack_dcmajor(a):
        # [1024, 256] -> [128, dc*1024 + c*128]
        return np.concatenate(
            [_pack_cmajor(a[:, 0:128]), _pack_cmajor(a[:, 128:256])], axis=1)

    in_maps = []
    for g in range(N_CORES):
        b, hg = divmod(g, 4)
        cs = slice(hg * DL, (hg + 1) * DL)
        wqh, wql = _split_fp8(WS * W_attn[:, 0 * C:1 * C][:, cs])
        wkh, wkl = _split_fp8(WS * W_attn[:, 1 * C:2 * C][:, cs])
        wvh, wvl = _split_fp8(WS * W_attn[:, 2 * C:3 * C][:, cs])
        wp = _pack_cmajor(W_proj[cs, :].astype(BF16))
        bq = np.ascontiguousarray(WS * b_attn[0 * C:1 * C][cs].reshape(2, 128).T)
        bk = np.ascontiguousarray(WS * b_attn[1 * C:2 * C][cs].reshape(2, 128).T)
        bvr = np.tile(WS * b_attn[2 * C:3 * C][cs][None, :], (128, 1))
        m32 = np.ascontiguousarray(
            np.concatenate([bq, bk, bvr], axis=1).astype(np.float32))
        in_maps.append({
            "xhp": xhl_b[b][0], "xlp": xhl_b[b][1],
            "wqh": _pack_dcmajor(wqh), "wql": _pack_dcmajor(wql),
            "wkh": _pack_dcmajor(wkh), "wkl": _pack_dcmajor(wkl),
            "wvh": _pack_cmajor(wvh), "wvl": _pack_cmajor(wvl),
            "wpp": wp, "m32": m32, "mbf": mbf,
        })
    return in_maps


def assemble_output(results, b_proj):
    """results: per-core dicts with 'out' [T, C] fp16 partials."""
    b_proj = np.asarray(b_proj, np.float32)
    out = np.zeros((B, T, C), np.float32)
    for g in range(N_CORES):
        out[g // 4] += np.asarray(results[g]["out"], np.float32)
    out += b_proj[None, None, :]
    return out


def kernel(x, W_attn, b_attn, W_proj, b_proj):
    from concourse.bass_utils import run_bass_kernel_spmd

    nc = get_program()
    in_maps = make_in_maps(x, W_attn, b_attn, W_proj)
    res = run_bass_kernel_spmd(nc, in_maps, list(range(N_CORES)))
    return assemble_output(res.results, b_proj)
